# revision 1
# baseline (speedup 1.0000x reference)
"""Multi-head causal attention (B=2, S=2048, D=1024, H=16) on 8 trn2 cores.

Sharding: core c -> (batch b = c//4, head-group g = c%4, 4 heads each).
Data-parallel over B, tensor-parallel over heads. Each core computes a
partial output projection [S, D]; the host sums the 4 partials per batch
and adds b_out.

Device kernel per core (all matmuls in float32r):
  A) qkT[f=512, s=2048] = (x @ Wqk)^T and v[s, f=256] = x @ Wv (+biases,
     folded in as K=1 rank-1 matmuls into PSUM).
     qkT feature layout: [K(h0)|K(h1)] [Q(h0)|Q(h1)] [K(h2)|K(h3)] [Q(h2)|Q(h3)]
  B) per head h, per 512-wide query block qmb: causal flash attention in
     the scores-TRANSPOSED layout: sT[k,q] = K @ Q^T so that attn@V is
     lhsT=v_blk[s,hd+1] (ones col appended -> softmax denominators in
     row 64 of PSUM), rhs=expT[k,q]. No on-chip transposes anywhere.
     Causal mask applied on the PE itself (psum += I.T @ trimask).
     The sc->exp->av chains are software-pipelined (av emission lags by
     `lag` links) and rotate through a deep PSUM pool so cross-engine
     semaphore wake latency is hidden.
  C) out_partial[s, 1024] = values^T.T @ W_out, DMA'd to DRAM.
"""
import math
import numpy as np

import concourse.bass as bass
import concourse.mybir as mybir
import concourse.tile as tile
from concourse import bacc
from concourse.bass_utils import run_bass_kernel_spmd

N_CORES = 8
B, S, D = 2, 2048, 1024
H = 16                    # total heads
HL = 4                    # heads per core
HD = 64                   # head dim
FQK = 2 * HL * HD         # 512 local q+k features
FV = HL * HD              # 256 local v features
SCALE = 1.0 / math.sqrt(HD)
NEG = -1e9

QMB = 512                 # query macro-block
KB = 128                  # key block
N_QMB = S // QMB          # 4
N_KB = S // KB            # 16

F32 = mybir.dt.float32
F32R = mybir.dt.float32r
BF16 = mybir.dt.bfloat16


def build_kernel(repeat: int = 1, stages: str = "ABC", xdma_in_loop: bool = True,
                 bmode: str = "full", pairw: int = 2, wave: int = 2,
                 sc_bufs: int = 3, av_bufs: int = 2, exp_bufs: int = 8,
                 lag: int = 2, fullexp: bool = False,
                 fill_first: bool = False):
    assert sc_bufs * pairw + av_bufs <= 8
    W = 512 * pairw
    nc = bacc.Bacc(
        "TRN2", target_bir_lowering=False, debug=False, num_devices=N_CORES
    )
    xT = nc.dram_tensor("xT", [D, S], F32R, kind="ExternalInput")
    wqk = nc.dram_tensor("wqk", [D, FQK], F32R, kind="ExternalInput")
    wv = nc.dram_tensor("wv", [D, FV], F32R, kind="ExternalInput")
    wo = nc.dram_tensor("wo", [FV, D], F32R, kind="ExternalInput")
    bqk = nc.dram_tensor("bqk", [FQK], F32R, kind="ExternalInput")
    bv = nc.dram_tensor("bv", [FV], F32R, kind="ExternalInput")
    out = nc.dram_tensor("out", [S, D], F32, kind="ExternalOutput")

    KT = D // 128  # 8 contraction tiles over D

    with tile.TileContext(nc) as tc:
        dma = nc.sync  # HWDGE: spreads transfers over HW queues
        with (
            tc.tile_pool(name="const", bufs=1) as const,
            tc.tile_pool(name="xt", bufs=1) as xtp,
            tc.tile_pool(name="big", bufs=1) as big,
            tc.tile_pool(name="exp", bufs=exp_bufs) as expp,
            tc.tile_pool(name="small", bufs=4) as small,
            tc.tile_pool(name="ob", bufs=3) as obp,
            tc.tile_pool(name="ps_sc", bufs=sc_bufs, space="PSUM") as ps_sc,
            tc.tile_pool(name="ps_av", bufs=av_bufs, space="PSUM") as ps_av,
        ):
            # ---- constants ----
            wqk_sb = const.tile([128, KT, FQK], F32R)
            wv_sb = const.tile([128, KT, FV], F32R)
            wo_sb = const.tile([128, FV // 128, D], F32R)
            dma.dma_start(
                out=wqk_sb, in_=wqk.rearrange("(kt p) f -> p kt f", p=128)
            )
            dma.dma_start(
                out=wv_sb, in_=wv.rearrange("(kt p) f -> p kt f", p=128)
            )
            dma.dma_start(
                out=wo_sb, in_=wo.rearrange("(dt p) f -> p dt f", p=128)
            )
            bqk_sb = const.tile([1, FQK], F32R)
            bv_sb = const.tile([1, FV], F32R)
            dma.dma_start(out=bqk_sb, in_=bqk.rearrange("(o f) -> o f", o=1))
            dma.dma_start(out=bv_sb, in_=bv.rearrange("(o f) -> o f", o=1))
            ones_f32 = const.tile([1, QMB], F32)
            nc.vector.memset(ones_f32, 1.0)
            ones_row = const.tile([1, QMB], F32R)
            nc.vector.tensor_copy(ones_row, ones_f32)
            # additive causal mask for the diagonal 128x128 block:
            # trimask[k, q] = 0 if k <= q else NEG  (bf16 for fast mask MMs)
            trimask = const.tile([128, 128], F32)
            nc.gpsimd.memset(trimask, 0.0)
            nc.gpsimd.affine_select(
                out=trimask,
                in_=trimask,
                compare_op=mybir.AluOpType.is_ge,
                fill=NEG,
                base=0,
                pattern=[[1, 128]],
                channel_multiplier=-1,
            )
            trimask_r = const.tile([128, 128], BF16)
            nc.vector.tensor_copy(trimask_r, trimask)
            ident_f32 = const.tile([128, 128], F32)
            nc.gpsimd.memset(ident_f32, 0.0)
            nc.gpsimd.affine_select(
                out=ident_f32,
                in_=ident_f32,
                compare_op=mybir.AluOpType.not_equal,
                fill=1.0,
                base=0,
                pattern=[[-1, 128]],
                channel_multiplier=1,
            )
            ident_r = const.tile([128, 128], BF16)
            nc.vector.tensor_copy(ident_r, ident_f32)

            # ---- persistent intermediates ----
            qkT = big.tile([128, 4, S], F32R)            # 4 f-tiles x S
            v_aug = big.tile([128, N_KB, HL, HD + 1], F32R)
            valuesT = big.tile([128, FV // 128, S], F32R)
            vone_f32 = const.tile([128, N_KB * HL], F32)
            nc.vector.memset(vone_f32, 1.0)
            nc.vector.tensor_copy(
                v_aug[:, :, :, HD:HD + 1],
                vone_f32.rearrange("p (kb h o) -> p kb h o", h=HL, o=1),
            )

            xts_full = None
            if not xdma_in_loop:
                xts_full = []
                for kt in range(KT):
                    xt_t = xtp.tile([128, S], F32R, tag=f"xt{kt}")
                    dma.dma_start(
                        out=xt_t, in_=xT[kt * 128:(kt + 1) * 128, :]
                    )
                    xts_full.append(xt_t)

            def body(_it):
                # ======== stage A: qkT and v_aug, in two column halves ====
                # Half 0 runs up front; half 1 is queued as PE-filler work
                # items dripped into B(qmb0/1), which only need half 0.
                def load_xts(half):
                    s0 = half * (S // 2)
                    if xdma_in_loop:
                        xts = []
                        for kt in range(KT):
                            xt_t = xtp.tile([128, S // 2], F32R, tag=f"xt{kt}")
                            dma.dma_start(
                                out=xt_t,
                                in_=xT[kt * 128:(kt + 1) * 128, s0:s0 + S // 2],
                            )
                            xts.append(xt_t)
                        return xts
                    return [t[:, s0:s0 + S // 2] for t in xts_full]

                def make_qk_item(xts, half, ft, nt):
                    s0 = half * (S // 2)

                    def emit():
                        c0 = nt * 512
                        ps = ps_sc.tile([128, W], F32, tag="sc")
                        for kt in range(KT):
                            nc.tensor.matmul(
                                ps[:, 0:512],
                                wqk_sb[:, kt, ft * 128:(ft + 1) * 128],
                                xts[kt][:, c0:c0 + 512],
                                start=(kt == 0),
                                stop=False,
                            )
                        nc.tensor.matmul(
                            ps[:, 0:512],
                            bqk_sb[0:1, ft * 128:(ft + 1) * 128],
                            ones_row,
                            start=False,
                            stop=True,
                        )
                        nc.vector.tensor_copy(
                            qkT[:, ft, s0 + c0:s0 + c0 + 512], ps[:, 0:512]
                        )
                    return emit

                def make_v_item(xts, half, stp):
                    def emit():
                        psv = ps_sc.tile([128, 512], F32, tag="sc")
                        for sub in range(2):
                            sti = stp * 2 + sub
                            c0 = sub * FV
                            for kt in range(KT):
                                nc.tensor.matmul(
                                    psv[:, c0:c0 + FV],
                                    xts[kt][:, sti * 128:(sti + 1) * 128],
                                    wv_sb[:, kt, :],
                                    start=(kt == 0),
                                    stop=False,
                                )
                            nc.tensor.matmul(
                                psv[:, c0:c0 + FV],
                                ones_row[0:1, 0:128],
                                bv_sb,
                                start=False,
                                stop=True,
                            )
                        st0 = half * 8 + stp * 2
                        nc.vector.tensor_copy(
                            v_aug[:, st0:st0 + 2, :, 0:HD],
                            psv.rearrange("s (t h c) -> s t h c", t=2, h=HL),
                        )
                    return emit

                def a_items(xts, half):
                    items = []
                    for ft in range(4):
                        for nt in range(2):
                            items.append(make_qk_item(xts, half, ft, nt))
                    for stp in range(4):
                        items.append(make_v_item(xts, half, stp))
                    return items

                xts0 = load_xts(0)
                for it in a_items(xts0, 0):
                    it()
                xts1 = load_xts(1)
                for it in a_items(xts1, 1):
                    it()
                filler = []

                if "B" not in stages:
                    # sink so DCE keeps stage A
                    dma.dma_start(
                        out=out[0:128, 0:512],
                        in_=qkT[:, 0, 0:512].bitcast(F32),
                    )
                    return

                # ======== stage B+C: per query macro-block ========
                # A-half1 and C work are drip-fed into B's matmul stream
                # as PE filler (keeps the PE HAM-warm).

                def make_c_item(st):
                    def emit():
                        ob = obp.tile([128, 1024], F32)
                        for nt in range(2):
                            ps = ps_sc.tile([128, W], F32, tag="sc")
                            for dt_ in range(FV // 128):
                                nc.tensor.matmul(
                                    ps[:, 0:512],
                                    valuesT[:, dt_, st * 128:(st + 1) * 128],
                                    wo_sb[:, dt_, nt * 512:(nt + 1) * 512],
                                    start=(dt_ == 0),
                                    stop=(dt_ == FV // 128 - 1),
                                )
                            nc.vector.tensor_copy(
                                ob[:, nt * 512:(nt + 1) * 512], ps[:, 0:512]
                            )
                        dma.dma_start(
                            out=out[st * 128:(st + 1) * 128, :], in_=ob
                        )
                    return emit

                for qmb in range(N_QMB):
                    if qmb == 2:
                        while filler:
                            filler.pop(0)()
                    q0 = qmb * QMB
                    nkb = 4 * qmb + 4
                    nblk = nkb // pairw
                    for w0 in range(0, HL, wave):
                        whs = list(range(w0, w0 + wave))
                        avs = {
                            h_: ps_av.tile([65, QMB], F32, tag="av",
                                           name=f"av{h_}")
                            for h_ in whs
                        }
                        avq = []

                        def emit_av(item):
                            h, mms = item
                            for kb, col0, avw, ex_t in mms:
                                nc.tensor.matmul(
                                    avs[h][0:65, col0:col0 + avw],
                                    v_aug[:, kb, h, :],
                                    ex_t,
                                    start=(kb == 0),
                                    stop=(kb == nkb - 1),
                                )

                        for blk in range(nblk):
                            kb0 = blk * pairw
                            diag = kb0 + pairw - 1 >= 4 * qmb
                            scs = {}
                            if wave == 2:
                                # row-packed: both heads' score MMs emitted
                                # back-to-back; lhsT base partitions 0/64 ->
                                # concurrent row-group execution on the PE.
                                for h in whs:
                                    scs[h] = ps_sc.tile(
                                        [128, W], F32, tag="sc",
                                        name=f"sc{h}"
                                    )
                                for sub in range(pairw):
                                    kb = kb0 + sub
                                    j = kb - 4 * qmb
                                    col0 = 128 * j if j >= 0 else 0
                                    cb = sub * 512 + col0
                                    scw = 512 - col0
                                    for h in whs:
                                        tk = 2 * (h // 2)
                                        pk = 64 * (h % 2)
                                        nc.tensor.matmul(
                                            scs[h][:, cb:cb + scw],
                                            qkT[pk:pk + 64, tk,
                                                kb * KB:(kb + 1) * KB],
                                            qkT[pk:pk + 64, tk + 1,
                                                q0 + col0:q0 + col0 + scw],
                                            start=True,
                                            stop=(j < 0),
                                            skip_group_check=True,
                                        )
                                if diag:
                                    for h in whs:
                                        for sub in range(pairw):
                                            j = kb0 + sub - 4 * qmb
                                            if j < 0:
                                                continue
                                            cb = sub * 512 + 128 * j
                                            nc.tensor.matmul(
                                                scs[h][:, cb:cb + 128],
                                                ident_r,
                                                trimask_r,
                                                start=False,
                                                stop=True,
                                                skip_group_check=True,
                                            )
                            for h in whs:
                                tk = 2 * (h // 2)
                                pk = 64 * (h % 2)
                                kT_h = qkT[pk:pk + 64, tk, :]
                                qT_h = qkT[pk:pk + 64, tk + 1, :]
                                if wave == 2:
                                    sc = scs[h]
                                else:
                                    sc = ps_sc.tile([128, W], F32, tag="sc")
                                ex = expp.tile([128, W], F32R)
                                if wave != 2:
                                    for sub in range(pairw):
                                        kb = kb0 + sub
                                        j = kb - 4 * qmb
                                        col0 = 128 * j if j >= 0 else 0
                                        cb = sub * 512 + col0
                                        scw = (64 if bmode == "tiny_sc"
                                               else 512 - col0)
                                        nc.tensor.matmul(
                                            sc[:, cb:cb + scw],
                                            kT_h[:, kb * KB:(kb + 1) * KB],
                                            qT_h[:, q0 + col0:
                                                 q0 + col0 + scw],
                                            start=True,
                                            stop=(j < 0 or bmode == "nomask"),
                                        )
                                        if j >= 0 and bmode != "nomask":
                                            # causal mask: += I.T@trimask
                                            nc.tensor.matmul(
                                                sc[:, cb:cb + 128],
                                                ident_r,
                                                trimask_r,
                                                start=False,
                                                stop=True,
                                            )
                                if bmode == "tiny_exp":
                                    nc.scalar.activation(
                                        out=ex[:, 0:64],
                                        in_=sc[:, 0:64],
                                        func=mybir.ActivationFunctionType.Exp,
                                        scale=SCALE,
                                    )
                                elif pairw == 1 or (diag and not fullexp):
                                    for sub in range(pairw):
                                        j = kb0 + sub - 4 * qmb
                                        col0 = 128 * j if j >= 0 else 0
                                        cb = sub * 512 + col0
                                        nc.scalar.activation(
                                            out=ex[:, cb:sub * 512 + 512],
                                            in_=sc[:, cb:sub * 512 + 512],
                                            func=(mybir
                                                  .ActivationFunctionType.Exp),
                                            scale=SCALE,
                                        )
                                else:
                                    # one full-tile exp even for diagonal
                                    # pairs: the below-diagonal garbage
                                    # region of ex is never read by the av
                                    # matmuls (they slice [col0:512]).
                                    nc.scalar.activation(
                                        out=ex,
                                        in_=sc,
                                        func=mybir.ActivationFunctionType.Exp,
                                        scale=SCALE,
                                    )
                                mms = []
                                for sub in range(pairw):
                                    kb = kb0 + sub
                                    j = kb - 4 * qmb
                                    col0 = 128 * j if j >= 0 else 0
                                    avw = (64 if bmode == "tiny_av"
                                           else QMB - col0)
                                    mms.append((
                                        kb, col0, avw,
                                        ex[:, sub * 512 + col0:
                                            sub * 512 + col0 + avw],
                                    ))
                                avq.append((h, mms))
                            if fill_first and filler:
                                filler.pop(0)()
                            while len(avq) > wave * lag:
                                emit_av(avq.pop(0))
                            if not fill_first and filler:
                                filler.pop(0)()
                        while avq:
                            emit_av(avq.pop(0))

                        # normalize: values = av[0:64] / av[64]
                        for h in whs:
                            av = avs[h]
                            if bmode == "notail":
                                snk = small.tile([1, QMB], F32, tag="snk")
                                nc.vector.tensor_copy(snk, av[64:65, :])
                                dma.dma_start(
                                    out=out[128 + h:129 + h, 0:QMB], in_=snk
                                )
                                continue
                            rec = small.tile([1, QMB], F32R, tag="rec")
                            with nc.allow_low_precision(
                                reason="softmax denom feeds f32r matmul"
                            ):
                                nc.vector.reciprocal(rec, av[64:65, :])
                            rb = small.tile([64, QMB], F32R, tag="rb")
                            nc.gpsimd.partition_broadcast(rb, rec)
                            dt_ = h // 2
                            pr = 64 * (h % 2)
                            nc.vector.tensor_mul(
                                valuesT[pr:pr + 64, dt_, q0:q0 + QMB],
                                av[0:64, :],
                                rb,
                            )
                    # ---- queue stage C for this qmb ----
                    if "C" not in stages:
                        if bmode != "notail":
                            dma.dma_start(
                                out=out[qmb * 128:(qmb + 1) * 128, 0:512],
                                in_=valuesT[:, 0, qmb * 512:qmb * 512 + 512]
                                .bitcast(F32),
                            )
                        continue
                    for sti in range(QMB // 128):
                        filler.append(make_c_item(qmb * 4 + sti))
                while filler:
                    filler.pop(0)()

            if repeat == 1:
                body(0)
            else:
                with tc.For_i(
                    0, repeat, 1,
                    hint_engines=(mybir.EngineType.PE,),
                ) as it:
                    body(it)
    nc.compile()
    return nc


def make_in_maps(x, W_qkv, b_qkv, W_out, b_out):
    """Host-side sharding: per-core input dict."""
    x = np.asarray(x, dtype=np.float32)
    W_qkv = np.asarray(W_qkv, dtype=np.float32)
    b_qkv = np.asarray(b_qkv, dtype=np.float32)
    W_out = np.asarray(W_out, dtype=np.float32)
    in_maps = []
    xT_by_b = [np.ascontiguousarray(x[b_].T) for b_ in range(B)]
    for c in range(N_CORES):
        b_ = c // 4
        g = c % 4
        heads = [4 * g + i for i in range(HL)]
        # feature order: K(h0),K(h1),Q(h0),Q(h1),K(h2),K(h3),Q(h2),Q(h3)
        qk_cols = []
        for pair in range(2):
            h0, h1 = heads[2 * pair], heads[2 * pair + 1]
            for h_ in (h0, h1):
                base = h_ * 3 * HD + 1 * HD  # K
                qk_cols.extend(range(base, base + HD))
            for h_ in (h0, h1):
                base = h_ * 3 * HD + 0 * HD  # Q
                qk_cols.extend(range(base, base + HD))
        v_cols = []
        for h_ in heads:
            base = h_ * 3 * HD + 2 * HD  # V
            v_cols.extend(range(base, base + HD))
        qk_cols = np.array(qk_cols)
        v_cols = np.array(v_cols)
        in_maps.append({
            "xT": xT_by_b[b_],
            "wqk": np.ascontiguousarray(W_qkv[:, qk_cols]),
            "wv": np.ascontiguousarray(W_qkv[:, v_cols]),
            "wo": np.ascontiguousarray(W_out[g * FV:(g + 1) * FV, :]),
            "bqk": np.ascontiguousarray(b_qkv[qk_cols]),
            "bv": np.ascontiguousarray(b_qkv[v_cols]),
        })
    return in_maps


_NC_CACHE = {}


def get_nc(repeat: int = 1):
    if repeat not in _NC_CACHE:
        _NC_CACHE[repeat] = build_kernel(repeat)
    return _NC_CACHE[repeat]


def kernel(x, W_qkv, b_qkv, W_out, b_out):
    in_maps = make_in_maps(x, W_qkv, b_qkv, W_out, b_out)
    nc = get_nc(1)
    res = run_bass_kernel_spmd(nc, in_maps, list(range(N_CORES)))
    b_out = np.asarray(b_out, dtype=np.float32)
    out = np.zeros((B, S, D), dtype=np.float32)
    for b_ in range(B):
        acc = np.zeros((S, D), dtype=np.float32)
        for g in range(4):
            acc += res.results[4 * b_ + g]["out"]
        out[b_] = acc + b_out[None, :]
    return out



# revision 3
# speedup vs baseline: 1.6167x; 1.6167x over previous
"""Multi-head causal attention (B=2, S=2048, D=1024, H=16) on 8 trn2 cores.

Sharding: core c -> (batch b = c//4, head-group g = c%4, 4 heads each).
Data-parallel over B, tensor-parallel over heads. Each core computes a
partial output projection [S, D]; the host sums the 4 partials per batch
and adds b_out.

v2: all operands bf16 (PSUM accumulation stays f32) — same PE rate as
f32r at 1 cycle/row, half the DMA traffic and SBUF footprint. The freed
SBUF double-buffers qkT / v_aug / valuesT / xt across repeat iterations,
and the repeat loop is 2x-unrolled so consecutive bodies ping-pong
buffers and overlap through plain dataflow deps (stage A of body k+1
runs during stage B/C of body k). staggered_reset avoids the all-engine
barrier at the loop back-edge.

Device kernel per core:
  A) qkT[f=512, s=2048] = (x @ Wqk)^T and v[s, f=256] = x @ Wv (+biases,
     folded in as rank-1 matmuls into PSUM).
  B) per head h, per 512-wide query block qmb: causal flash attention in
     the scores-TRANSPOSED layout: sT[k,q] = K @ Q^T so that attn@V is
     lhsT=v_blk[s,hd+1] (ones col appended -> softmax denominators in
     row 64 of PSUM), rhs=expT[k,q]. Causal mask applied on the PE
     (psum += I.T @ trimask). sc->exp->av chains are software-pipelined.
  C) out_partial[s, 1024] = values^T.T @ W_out, DMA'd to DRAM (bf16).
"""
import math
import numpy as np
import ml_dtypes

import concourse.bass as bass
import concourse.mybir as mybir
import concourse.tile as tile
from concourse import bacc
from concourse.bass_utils import run_bass_kernel_spmd

N_CORES = 8
B, S, D = 2, 2048, 1024
H = 16                    # total heads
HL = 4                    # heads per core
HD = 64                   # head dim
FQK = 2 * HL * HD         # 512 local q+k features
FV = HL * HD              # 256 local v features
SCALE = 1.0 / math.sqrt(HD)
NEG = -1e9

QMB = 512                 # query macro-block
KB = 128                  # key block
N_QMB = S // QMB          # 4
N_KB = S // KB            # 16

F32 = mybir.dt.float32
F32R = mybir.dt.float32r
BF16 = mybir.dt.bfloat16
NP_BF16 = ml_dtypes.bfloat16


def build_kernel(repeat: int = 1, stages: str = "ABC",
                 bmode: str = "full", pairw: int = 2, wave: int = 2,
                 sc_bufs: int = 3, av_bufs: int = 2, exp_bufs: int = 8,
                 lag: int = 2, fullexp: bool = False,
                 fill_first: bool = False, unroll: int = 2,
                 staggered: bool = True):
    assert sc_bufs * pairw + av_bufs <= 8
    W = 512 * pairw
    nc = bacc.Bacc(
        "TRN2", target_bir_lowering=False, debug=False, num_devices=N_CORES
    )
    xT = nc.dram_tensor("xT", [D, S], BF16, kind="ExternalInput")
    wqk = nc.dram_tensor("wqk", [D, FQK], BF16, kind="ExternalInput")
    wv = nc.dram_tensor("wv", [D, FV], BF16, kind="ExternalInput")
    wo = nc.dram_tensor("wo", [FV, D], BF16, kind="ExternalInput")
    bqk = nc.dram_tensor("bqk", [FQK], BF16, kind="ExternalInput")
    bv = nc.dram_tensor("bv", [FV], BF16, kind="ExternalInput")
    out = nc.dram_tensor("out", [S, D], BF16, kind="ExternalOutput")

    KT = D // 128  # 8 contraction tiles over D

    with tile.TileContext(nc) as tc:
        dma = nc.sync  # HWDGE: spreads transfers over HW queues
        with (
            tc.tile_pool(name="const", bufs=1) as const,
            tc.tile_pool(name="xt", bufs=2) as xtp,
            tc.tile_pool(name="pp", bufs=2) as pp,
            tc.tile_pool(name="exp", bufs=exp_bufs) as expp,
            tc.tile_pool(name="small", bufs=4) as small,
            tc.tile_pool(name="ob", bufs=3) as obp,
            tc.tile_pool(name="ps_sc", bufs=sc_bufs, space="PSUM") as ps_sc,
            tc.tile_pool(name="ps_av", bufs=av_bufs, space="PSUM") as ps_av,
        ):
            # ---- constants ----
            wqk_sb = const.tile([128, KT, FQK], BF16)
            wv_sb = const.tile([128, KT, FV], BF16)
            wo_sb = const.tile([128, FV // 128, D], BF16)
            wqk_r = wqk.rearrange("(kt p) f -> p kt f", p=128)
            for kt in range(KT):
                dma.dma_start(
                    out=wqk_sb[:, kt:kt + 1, :], in_=wqk_r[:, kt:kt + 1, :]
                )
            dma.dma_start(
                out=wv_sb, in_=wv.rearrange("(kt p) f -> p kt f", p=128)
            )
            dma.dma_start(
                out=wo_sb, in_=wo.rearrange("(dt p) f -> p dt f", p=128)
            )
            bqk_sb = const.tile([1, FQK], BF16)
            bv_sb = const.tile([1, FV], BF16)
            dma.dma_start(out=bqk_sb, in_=bqk.rearrange("(o f) -> o f", o=1))
            dma.dma_start(out=bv_sb, in_=bv.rearrange("(o f) -> o f", o=1))
            ones_f32 = const.tile([1, QMB], F32)
            nc.vector.memset(ones_f32, 1.0)
            ones_row = const.tile([1, QMB], BF16)
            nc.vector.tensor_copy(ones_row, ones_f32)
            # additive causal mask for the diagonal 128x128 block:
            # trimask[k, q] = 0 if k <= q else NEG  (bf16 for fast mask MMs)
            trimask = const.tile([128, 128], F32)
            nc.gpsimd.memset(trimask, 0.0)
            nc.gpsimd.affine_select(
                out=trimask,
                in_=trimask,
                compare_op=mybir.AluOpType.is_ge,
                fill=NEG,
                base=0,
                pattern=[[1, 128]],
                channel_multiplier=-1,
            )
            trimask_r = const.tile([128, 128], BF16)
            nc.vector.tensor_copy(trimask_r, trimask)
            ident_f32 = const.tile([128, 128], F32)
            nc.gpsimd.memset(ident_f32, 0.0)
            nc.gpsimd.affine_select(
                out=ident_f32,
                in_=ident_f32,
                compare_op=mybir.AluOpType.not_equal,
                fill=1.0,
                base=0,
                pattern=[[-1, 128]],
                channel_multiplier=1,
            )
            ident_r = const.tile([128, 128], BF16)
            nc.vector.tensor_copy(ident_r, ident_f32)
            vone_f32 = const.tile([128, N_KB * HL], F32)
            nc.vector.memset(vone_f32, 1.0)

            def body(_it):
                # double-buffered across bodies: consecutive bodies use
                # alternate buffers (pp/xt pools, bufs=2), so body k+1's
                # stage A overlaps body k's B/C via plain dataflow deps.
                qkT = pp.tile([128, 4, S], BF16, tag="qkT")
                v_aug = pp.tile([128, N_KB, HL, HD + 1], BF16, tag="vaug")
                valuesT = pp.tile([128, FV // 128, S], BF16, tag="valT")
                nc.vector.tensor_copy(
                    v_aug[:, :, :, HD:HD + 1],
                    vone_f32.rearrange("p (kb h o) -> p kb h o", h=HL, o=1),
                )

                # ======== stage A: qkT and v_aug ====
                # Both x halves are DMA'd up front (xt pool is
                # double-buffered); half-1 compute items are queued as
                # PE-filler work dripped into B(qmb0/1), which only need
                # half 0.
                def load_xts(half):
                    s0 = half * (S // 2)
                    xts = []
                    for kt in range(KT):
                        xt_t = xtp.tile([128, S // 2], BF16, tag=f"xt{kt}")
                        dma.dma_start(
                            out=xt_t,
                            in_=xT[kt * 128:(kt + 1) * 128, s0:s0 + S // 2],
                        )
                        xts.append(xt_t)
                    return xts

                def make_qk_item(xts, half, ft, nt):
                    s0 = half * (S // 2)

                    def emit():
                        c0 = nt * 512
                        ps = ps_sc.tile([128, W], F32, tag="sc")
                        for kt in range(KT):
                            nc.tensor.matmul(
                                ps[:, 0:512],
                                wqk_sb[:, kt, ft * 128:(ft + 1) * 128],
                                xts[kt][:, c0:c0 + 512],
                                start=(kt == 0),
                                stop=False,
                            )
                        nc.tensor.matmul(
                            ps[:, 0:512],
                            bqk_sb[0:1, ft * 128:(ft + 1) * 128],
                            ones_row,
                            start=False,
                            stop=True,
                        )
                        nc.vector.tensor_copy(
                            qkT[:, ft, s0 + c0:s0 + c0 + 512], ps[:, 0:512]
                        )
                    return emit

                def make_v_item(xts, half, stp):
                    def emit():
                        psv = ps_sc.tile([128, 512], F32, tag="sc")
                        for sub in range(2):
                            sti = stp * 2 + sub
                            c0 = sub * FV
                            for kt in range(KT):
                                nc.tensor.matmul(
                                    psv[:, c0:c0 + FV],
                                    xts[kt][:, sti * 128:(sti + 1) * 128],
                                    wv_sb[:, kt, :],
                                    start=(kt == 0),
                                    stop=False,
                                )
                            nc.tensor.matmul(
                                psv[:, c0:c0 + FV],
                                ones_row[0:1, 0:128],
                                bv_sb,
                                start=False,
                                stop=True,
                            )
                        st0 = half * 8 + stp * 2
                        nc.vector.tensor_copy(
                            v_aug[:, st0:st0 + 2, :, 0:HD],
                            psv.rearrange("s (t h c) -> s t h c", t=2, h=HL),
                        )
                    return emit

                def a_items(xts, half):
                    items = []
                    for ft in range(4):
                        for nt in range(2):
                            items.append(make_qk_item(xts, half, ft, nt))
                    for stp in range(4):
                        items.append(make_v_item(xts, half, stp))
                    return items

                xts0 = load_xts(0)
                xts1 = load_xts(1)
                for it in a_items(xts0, 0):
                    it()
                filler = list(a_items(xts1, 1))

                if "B" not in stages:
                    for it in filler:
                        it()
                    dma.dma_start(
                        out=out[0:128, 0:512],
                        in_=qkT[:, 0, 0:512],
                    )
                    return

                # ======== stage B+C: per query macro-block ========
                # A-half1 and C work are drip-fed into B's matmul stream
                # as PE filler (keeps the PE HAM-warm).

                def make_c_item(st):
                    def emit():
                        ob = obp.tile([128, 1024], BF16)
                        for nt in range(2):
                            ps = ps_sc.tile([128, W], F32, tag="sc")
                            for dt_ in range(FV // 128):
                                nc.tensor.matmul(
                                    ps[:, 0:512],
                                    valuesT[:, dt_, st * 128:(st + 1) * 128],
                                    wo_sb[:, dt_, nt * 512:(nt + 1) * 512],
                                    start=(dt_ == 0),
                                    stop=(dt_ == FV // 128 - 1),
                                )
                            nc.vector.tensor_copy(
                                ob[:, nt * 512:(nt + 1) * 512], ps[:, 0:512]
                            )
                        dma.dma_start(
                            out=out[st * 128:(st + 1) * 128, :], in_=ob
                        )
                    return emit

                for qmb in range(N_QMB):
                    if qmb == 2:
                        while filler:
                            filler.pop(0)()
                    q0 = qmb * QMB
                    nkb = 4 * qmb + 4
                    nblk = nkb // pairw
                    for w0 in range(0, HL, wave):
                        whs = list(range(w0, w0 + wave))
                        avs = {
                            h_: ps_av.tile([65, QMB], F32, tag="av",
                                           name=f"av{h_}")
                            for h_ in whs
                        }
                        avq = []

                        def emit_av(item):
                            h, mms = item
                            for kb, col0, avw, ex_t in mms:
                                nc.tensor.matmul(
                                    avs[h][0:65, col0:col0 + avw],
                                    v_aug[:, kb, h, :],
                                    ex_t,
                                    start=(kb == 0),
                                    stop=(kb == nkb - 1),
                                )

                        for blk in range(nblk):
                            kb0 = blk * pairw
                            diag = kb0 + pairw - 1 >= 4 * qmb
                            scs = {}
                            if wave == 2:
                                # row-packed: both heads' score MMs emitted
                                # back-to-back; lhsT base partitions 0/64 ->
                                # concurrent row-group execution on the PE.
                                for h in whs:
                                    scs[h] = ps_sc.tile(
                                        [128, W], F32, tag="sc",
                                        name=f"sc{h}"
                                    )
                                for sub in range(pairw):
                                    kb = kb0 + sub
                                    j = kb - 4 * qmb
                                    col0 = 128 * j if j >= 0 else 0
                                    cb = sub * 512 + col0
                                    scw = 512 - col0
                                    for h in whs:
                                        tk = 2 * (h // 2)
                                        pk = 64 * (h % 2)
                                        nc.tensor.matmul(
                                            scs[h][:, cb:cb + scw],
                                            qkT[pk:pk + 64, tk,
                                                kb * KB:(kb + 1) * KB],
                                            qkT[pk:pk + 64, tk + 1,
                                                q0 + col0:q0 + col0 + scw],
                                            start=True,
                                            stop=(j < 0),
                                            skip_group_check=True,
                                        )
                                if diag:
                                    for h in whs:
                                        for sub in range(pairw):
                                            j = kb0 + sub - 4 * qmb
                                            if j < 0:
                                                continue
                                            cb = sub * 512 + 128 * j
                                            nc.tensor.matmul(
                                                scs[h][:, cb:cb + 128],
                                                ident_r,
                                                trimask_r,
                                                start=False,
                                                stop=True,
                                                skip_group_check=True,
                                            )
                            for h in whs:
                                tk = 2 * (h // 2)
                                pk = 64 * (h % 2)
                                kT_h = qkT[pk:pk + 64, tk, :]
                                qT_h = qkT[pk:pk + 64, tk + 1, :]
                                if wave == 2:
                                    sc = scs[h]
                                else:
                                    sc = ps_sc.tile([128, W], F32, tag="sc")
                                ex = expp.tile([128, W], BF16)
                                if wave != 2:
                                    for sub in range(pairw):
                                        kb = kb0 + sub
                                        j = kb - 4 * qmb
                                        col0 = 128 * j if j >= 0 else 0
                                        cb = sub * 512 + col0
                                        scw = (64 if bmode == "tiny_sc"
                                               else 512 - col0)
                                        nc.tensor.matmul(
                                            sc[:, cb:cb + scw],
                                            kT_h[:, kb * KB:(kb + 1) * KB],
                                            qT_h[:, q0 + col0:
                                                 q0 + col0 + scw],
                                            start=True,
                                            stop=(j < 0 or bmode == "nomask"),
                                        )
                                        if j >= 0 and bmode != "nomask":
                                            # causal mask: += I.T@trimask
                                            nc.tensor.matmul(
                                                sc[:, cb:cb + 128],
                                                ident_r,
                                                trimask_r,
                                                start=False,
                                                stop=True,
                                            )
                                if bmode == "tiny_exp":
                                    nc.scalar.activation(
                                        out=ex[:, 0:64],
                                        in_=sc[:, 0:64],
                                        func=mybir.ActivationFunctionType.Exp,
                                        scale=SCALE,
                                    )
                                elif pairw == 1 or (diag and not fullexp):
                                    for sub in range(pairw):
                                        j = kb0 + sub - 4 * qmb
                                        col0 = 128 * j if j >= 0 else 0
                                        cb = sub * 512 + col0
                                        nc.scalar.activation(
                                            out=ex[:, cb:sub * 512 + 512],
                                            in_=sc[:, cb:sub * 512 + 512],
                                            func=(mybir
                                                  .ActivationFunctionType.Exp),
                                            scale=SCALE,
                                        )
                                else:
                                    # one full-tile exp even for diagonal
                                    # pairs: the below-diagonal garbage
                                    # region of ex is never read by the av
                                    # matmuls (they slice [col0:512]).
                                    nc.scalar.activation(
                                        out=ex,
                                        in_=sc,
                                        func=mybir.ActivationFunctionType.Exp,
                                        scale=SCALE,
                                    )
                                mms = []
                                for sub in range(pairw):
                                    kb = kb0 + sub
                                    j = kb - 4 * qmb
                                    col0 = 128 * j if j >= 0 else 0
                                    avw = (64 if bmode == "tiny_av"
                                           else QMB - col0)
                                    mms.append((
                                        kb, col0, avw,
                                        ex[:, sub * 512 + col0:
                                            sub * 512 + col0 + avw],
                                    ))
                                avq.append((h, mms))
                            if fill_first and filler:
                                filler.pop(0)()
                            while len(avq) > wave * lag:
                                emit_av(avq.pop(0))
                            if not fill_first and filler:
                                filler.pop(0)()
                        while avq:
                            emit_av(avq.pop(0))

                        # normalize: values = av[0:64] / av[64]
                        for h in whs:
                            av = avs[h]
                            if bmode == "notail":
                                snk = small.tile([1, QMB], F32, tag="snk")
                                nc.vector.tensor_copy(snk, av[64:65, :])
                                dma.dma_start(
                                    out=out[128 + h:129 + h, 0:QMB], in_=snk
                                )
                                continue
                            rec = small.tile([1, QMB], F32R, tag="rec")
                            with nc.allow_low_precision(
                                reason="softmax denom feeds bf16 matmul"
                            ):
                                nc.vector.reciprocal(rec, av[64:65, :])
                            rb = small.tile([64, QMB], F32R, tag="rb")
                            nc.gpsimd.partition_broadcast(rb, rec)
                            dt_ = h // 2
                            pr = 64 * (h % 2)
                            with nc.allow_low_precision(
                                reason="attn values stored bf16"
                            ):
                                nc.vector.tensor_mul(
                                    valuesT[pr:pr + 64, dt_, q0:q0 + QMB],
                                    av[0:64, :],
                                    rb,
                                )
                    # ---- queue stage C for this qmb ----
                    if "C" not in stages:
                        if bmode != "notail":
                            dma.dma_start(
                                out=out[qmb * 128:(qmb + 1) * 128, 0:512],
                                in_=valuesT[:, 0, qmb * 512:qmb * 512 + 512],
                            )
                        continue
                    for sti in range(QMB // 128):
                        filler.append(make_c_item(qmb * 4 + sti))
                while filler:
                    filler.pop(0)()

            if repeat == 1:
                body(0)
            else:
                n_loop = repeat // unroll
                rem = repeat - n_loop * unroll
                if n_loop > 0:
                    with tc.For_i(
                        0, n_loop, 1,
                        hint_engines=(mybir.EngineType.PE,),
                        staggered_reset=staggered,
                    ) as it:
                        for _u in range(unroll):
                            body(it)
                for _u in range(rem):
                    body(0)
    nc.compile()
    return nc


def make_in_maps(x, W_qkv, b_qkv, W_out, b_out):
    """Host-side sharding: per-core input dict (bf16)."""
    x = np.asarray(x, dtype=np.float32)
    W_qkv = np.asarray(W_qkv, dtype=np.float32)
    b_qkv = np.asarray(b_qkv, dtype=np.float32)
    W_out = np.asarray(W_out, dtype=np.float32)
    in_maps = []
    xT_by_b = [
        np.ascontiguousarray(x[b_].T.astype(NP_BF16)) for b_ in range(B)
    ]
    for c in range(N_CORES):
        b_ = c // 4
        g = c % 4
        heads = [4 * g + i for i in range(HL)]
        # feature order: K(h0),K(h1),Q(h0),Q(h1),K(h2),K(h3),Q(h2),Q(h3)
        qk_cols = []
        for pair in range(2):
            h0, h1 = heads[2 * pair], heads[2 * pair + 1]
            for h_ in (h0, h1):
                base = h_ * 3 * HD + 1 * HD  # K
                qk_cols.extend(range(base, base + HD))
            for h_ in (h0, h1):
                base = h_ * 3 * HD + 0 * HD  # Q
                qk_cols.extend(range(base, base + HD))
        v_cols = []
        for h_ in heads:
            base = h_ * 3 * HD + 2 * HD  # V
            v_cols.extend(range(base, base + HD))
        qk_cols = np.array(qk_cols)
        v_cols = np.array(v_cols)
        in_maps.append({
            "xT": xT_by_b[b_],
            "wqk": np.ascontiguousarray(W_qkv[:, qk_cols].astype(NP_BF16)),
            "wv": np.ascontiguousarray(W_qkv[:, v_cols].astype(NP_BF16)),
            "wo": np.ascontiguousarray(
                W_out[g * FV:(g + 1) * FV, :].astype(NP_BF16)
            ),
            "bqk": np.ascontiguousarray(b_qkv[qk_cols].astype(NP_BF16)),
            "bv": np.ascontiguousarray(b_qkv[v_cols].astype(NP_BF16)),
        })
    return in_maps


_NC_CACHE = {}


def get_nc(repeat: int = 1):
    if repeat not in _NC_CACHE:
        _NC_CACHE[repeat] = build_kernel(repeat)
    return _NC_CACHE[repeat]


def kernel(x, W_qkv, b_qkv, W_out, b_out):
    in_maps = make_in_maps(x, W_qkv, b_qkv, W_out, b_out)
    nc = get_nc(1)
    res = run_bass_kernel_spmd(nc, in_maps, list(range(N_CORES)))
    b_out = np.asarray(b_out, dtype=np.float32)
    out = np.zeros((B, S, D), dtype=np.float32)
    for b_ in range(B):
        acc = np.zeros((S, D), dtype=np.float32)
        for g in range(4):
            acc += np.asarray(res.results[4 * b_ + g]["out"], dtype=np.float32)
        out[b_] = acc + b_out[None, :]
    return out
